# revision 21
# baseline (speedup 1.0000x reference)
"""Trainium2 Bass kernel for nn_MultiHeadAttention_26259430048704.

Multi-head attention with additive bias and a multiplicative "explored" mask
applied to the scores before softmax (masked scores are set to 0, so they
contribute exp(0)=1 to the softmax).

Sharding: 16 heads / 8 cores = 2 heads per core (tensor parallel over heads).
Each core computes projections for its 128 W-columns and full attention for
its 2 heads; the host concatenates the per-core transposed outputs.

Kernel structure (v4):
  - bias folded multiplicatively on the host: EBKm1 = keep*exp(bias^T) - 1
    in fp8e4m3 (masked entries = -1.0 exactly).  On device:
    et = exp(qk); et = et*(EBKm1+1) + (EBKm1==-1)  via ONE custom fused DVE
    op.  No identity-bias matmuls on the PE, no u8 mask / copy_predicated,
    and the bias stream is 1 byte/elem.
  - QK is K=64 per head at base partitions 0/64 -> the two heads run as
    concurrent PE row-tiles.  pV is likewise split into two concurrent K=64
    row-tiles per head, accumulating in separate psum banks merged by the
    normalization (ACT copy + DVE add).
  - softmax denominator: vhp column 0 is ones so Z = pout row 0; 1/Z via
    DVE reciprocal_approx_fast, replicated across partitions by gpsimd
    partition_broadcast (idle engine), then one DVE multiply.
  - The x^T inputs live in three persistent SBUF tiles; each loop iteration
    re-DMAs all 12MB for the NEXT iteration, interleaved one 256KB chunk per
    attention tile, so the bulk load never queues ahead of the per-tile EBK
    DMAs (which starved the pipeline by ~30us/iter when batched).
  - The timing build unrolls TWO bodies per For_i iteration so double
    buffered pools (projections, vhp) truly ping-pong across iterations
    (static emission pins pool slots; an unrolled pair alternates them).
"""

import sys

for _p in ("/opt/trn_rl_repo", "/root/.axon_site/_ro/trn_rl_repo"):
    if _p not in sys.path:
        sys.path.insert(0, _p)

import numpy as np
import ml_dtypes

BF16 = ml_dtypes.bfloat16
FP8 = ml_dtypes.float8_e4m3

N = 2048
HID = 1024
HEADS = 16
DK = 64
NCORES = 8
HPC = HEADS // NCORES  # 2 heads per core
DC = HPC * DK  # 128 output columns per core
KT = HID // 128  # 8 contraction tiles
MT = N // 128  # 16 m tiles
NCH = N // 512  # 4 n chunks

_cache = {}

CFG = {
    "pv_split": True,      # split pV into two concurrent K=64 row-tiles
    "proj_on_act": True,   # projection bias-add on ACT (else DVE)
    "vhp_on_act": False,   # vhp extraction copies on ACT (else DVE)
    "pending_depth": 4,    # software pipeline depth for pV
    "bias_fp8": True,      # EBK-1 in fp8e4m3 (else EBK in bf16)
}

_OPS_REGISTERED = {}


def _register_op(name, spec):
    import concourse.dve_ops as dve_ops
    from concourse.dve_ops import DveOp
    from concourse.dve_table_gen import dve_ver_for
    from concourse.dve_uop import DveOpSpec
    from concourse.dve_spec import lower

    if name in _OPS_REGISTERED:
        return _OPS_REGISTERED[name]
    for op in dve_ops.OPS:
        if op.name == name:
            _OPS_REGISTERED[name] = op
            return op
    ver = dve_ver_for("TRN2")
    opcode = max(dve_ops._SUB_OPCODE_FOR_NAME.values()) + 1
    sha = DveOpSpec(
        name=name, opcode=opcode, uops=lower(spec, ver=ver), rd1_en=True
    ).sha(ver)
    op = DveOp(name, spec, subdim=False, uops_sha={ver: sha})
    dve_ops.OPS.append(op)
    dve_ops.CUSTOM_DVE_SPECS[name] = spec
    dve_ops._SUB_OPCODE_FOR_NAME[name] = opcode
    _OPS_REGISTERED[name] = op
    return op


def _get_masked_mul_op():
    """bf16 variant: out = in0*in1 + (in1==0)."""
    from concourse.dve_spec import Spec, Src0, Src1, Zero, eq

    def _ref(in0, in1, c0, c1, c2):
        a = np.asarray(in0, np.float32)
        b = np.asarray(in1, np.float32)
        return a * b + (b == 0).astype(np.float32)

    return _register_op("MASKED_MUL_ANT", Spec(body=Src0 * Src1 + eq(Src1, Zero), reference=_ref))


def _get_masked_mul_m1_op():
    """fp8 variant: out = in0*(in1+1) + (in1==s0); called with s0=-1."""
    from concourse.dve_spec import Spec, Src0, Src1, One, C0, eq

    def _ref(in0, in1, c0, c1, c2):
        a = np.asarray(in0, np.float32)
        b = np.asarray(in1, np.float32)
        c0v = c0 if isinstance(c0, float) else np.asarray(c0, np.float32)
        return a * (b + 1.0) + (b == c0v).astype(np.float32)

    return _register_op(
        "MASKED_MUL_M1_ANT", Spec(body=Src0 * (Src1 + One) + eq(Src1, C0), reference=_ref)
    )


def _build(repeat=1):
    import concourse.bass as bass
    import concourse.bacc as bacc
    import concourse.mybir as mybir
    import concourse.tile as tile
    from concourse.masks import make_identity

    f32 = mybir.dt.float32
    bf16 = mybir.dt.bfloat16
    AF = mybir.ActivationFunctionType
    bias_dt = mybir.dt.float8e4 if CFG["bias_fp8"] else bf16
    mm_op = _get_masked_mul_m1_op() if CFG["bias_fp8"] else _get_masked_mul_op()

    nc = bacc.Bacc("TRN2", target_bir_lowering=False, debug=False)

    xts = {t: nc.dram_tensor(f"xT{t}", [HID, N], bf16, kind="ExternalInput") for t in "qkv"}
    Ws = {t: nc.dram_tensor(f"W{t}", [128, KT * DC], bf16, kind="ExternalInput") for t in "qkv"}
    bs = {t: nc.dram_tensor(f"b{t}", [DC, 1], f32, kind="ExternalInput") for t in "qkv"}
    ebkT = nc.dram_tensor("ebkT", [HPC * N, N], bias_dt, kind="ExternalInput")
    outT = nc.dram_tensor("outT", [DC, N], f32, kind="ExternalOutput")

    with tile.TileContext(nc) as tc:
        with (
            tc.tile_pool(name="constp", bufs=1) as constp,
            tc.tile_pool(name="xtp", bufs=1) as xtp,
            tc.tile_pool(name="pers", bufs=2) as pers,
            tc.tile_pool(name="biasp", bufs=12) as biasp,
            tc.tile_pool(name="ep", bufs=6) as ep,
            tc.tile_pool(name="normp", bufs=4) as normp,
            tc.tile_pool(name="outp", bufs=4) as outp,
            tc.tile_pool(name="projp", bufs=1, space="PSUM") as projp,
            tc.tile_pool(name="pspool", bufs=3, space="PSUM") as pspool,
            tc.tile_pool(name="popool", bufs=4, space="PSUM") as popool,
        ):
            ident = constp.tile([128, 128], bf16)
            make_identity(nc, ident)

            W_sb = {}
            b_sb = {}
            for t in "kvq":
                W_sb[t] = constp.tile([128, KT, DC], bf16, tag=f"w{t}", name=f"W{t}_sb")
                nc.sync.dma_start(
                    out=W_sb[t], in_=Ws[t].ap().rearrange("p (kt m) -> p kt m", kt=KT)
                )
                b_sb[t] = constp.tile([DC, 1], f32, tag=f"b{t}", name=f"b{t}_sb")
                nc.sync.dma_start(out=b_sb[t], in_=bs[t].ap())

            def emit_xt_dma(tile_, t, chunk):
                kt, ch2 = divmod(chunk, 2)
                xt_dram = xts[t].ap().rearrange("(kt p) n -> p kt n", p=128)
                nc.sync.dma_start(
                    out=tile_[:, kt, ch2 * 1024 : (ch2 + 1) * 1024],
                    in_=xt_dram[:, kt, ch2 * 1024 : (ch2 + 1) * 1024],
                )

            # persistent x^T tiles: the prologue fills them; each body re-DMAs
            # them in place (write-after-read deps keep this safe) so the 12MB
            # stream overlaps the previous body's attention phase.
            xt0 = {}
            for t in "kvq":
                xt0[t] = xtp.tile([128, KT, N], bf16, tag=f"xt{t}", name=f"xt_{t}")
                for c in range(16):
                    emit_xt_dma(xt0[t], t, c)

            def emit_body(xt_cur, prefetch):
                """One full kernel pass; reads xt_cur, optionally emits the
                next iteration's xt DMAs interleaved into the attention loop.
                Returns the tiles holding the next iteration's x^T."""
                # ---- Phase 1: projections ----
                proj = {}
                for t in "kvq":
                    proj[t] = pers.tile([128, N], bf16, tag=f"proj{t}", name=f"proj{t}_sb")
                    for ch in range(NCH):
                        ps = projp.tile([128, 512], f32, tag="pp", name="proj_ps")
                        for kt in range(KT):
                            nc.tensor.matmul(
                                ps,
                                lhsT=W_sb[t][:, kt, :],
                                rhs=xt_cur[t][:, kt, ch * 512 : (ch + 1) * 512],
                                start=(kt == 0),
                                stop=(kt == KT - 1),
                            )
                        if CFG["proj_on_act"]:
                            nc.scalar.activation(
                                proj[t][:, ch * 512 : (ch + 1) * 512],
                                ps,
                                AF.Identity,
                                bias=b_sb[t],
                            )
                        else:
                            nc.vector.tensor_scalar_add(
                                proj[t][:, ch * 512 : (ch + 1) * 512], ps, b_sb[t]
                            )

                # vh' per head: [m-part, mt, 65]; col 0 = ones (softmax denom
                # lands on psum partition 0, where gpsimd can broadcast from)
                vhp = [
                    pers.tile([128, MT, DK + 1], bf16, tag=f"vhp{h}", name=f"vhp{h}_sb")
                    for h in range(HPC)
                ]
                for h in range(HPC):
                    nc.vector.memset(vhp[h][:, :, 0:1], 1.0)
                for mb in range(MT):
                    pstr = projp.tile([128, 128], bf16, tag="pp", name="tr_ps")
                    nc.tensor.transpose(
                        pstr, proj["v"][:, mb * 128 : (mb + 1) * 128], ident
                    )
                    for h in range(HPC):
                        if CFG["vhp_on_act"]:
                            nc.scalar.copy(
                                vhp[h][:, mb, 1 : DK + 1], pstr[:, h * DK : (h + 1) * DK]
                            )
                        else:
                            nc.vector.tensor_copy(
                                vhp[h][:, mb, 1 : DK + 1], pstr[:, h * DK : (h + 1) * DK]
                            )

                # ---- Phase 2: attention ----
                state = {"pouts": None}
                pending = []

                def emit_pv(item):
                    et3, m, n0_, pouts_ = item
                    for h in range(HPC):
                        if CFG["pv_split"]:
                            nc.tensor.matmul(
                                pouts_[h][0][0 : DK + 1, :],
                                lhsT=vhp[h][0:64, m, :],
                                rhs=et3[0:64, h, :],
                                start=(m == 0),
                                stop=(m == MT - 1),
                            )
                            nc.tensor.matmul(
                                pouts_[h][1][0 : DK + 1, :],
                                lhsT=vhp[h][64:128, m, :],
                                rhs=et3[64:128, h, :],
                                start=(m == 0),
                                stop=(m == MT - 1),
                            )
                        else:
                            nc.tensor.matmul(
                                pouts_[h][0][0 : DK + 1, :],
                                lhsT=vhp[h][:, m, :],
                                rhs=et3[:, h, :],
                                start=(m == 0),
                                stop=(m == MT - 1),
                            )
                    if m == MT - 1:
                        emit_norm(n0_, pouts_)

                def emit_norm(n0_, pouts_):
                    for h in range(HPC):
                        raw = normp.tile([128, 512], f32, tag="raw", name="raw_t")
                        if CFG["pv_split"]:
                            # DVE may read only one PSUM operand: stage half A
                            # through the ACT engine, then add half B on DVE.
                            nc.scalar.copy(
                                raw[0 : DK + 1, :], pouts_[h][0][0 : DK + 1, :]
                            )
                            nc.vector.tensor_add(
                                raw[0 : DK + 1, :],
                                raw[0 : DK + 1, :],
                                pouts_[h][1][0 : DK + 1, :],
                            )
                        else:
                            nc.vector.tensor_copy(
                                raw[0 : DK + 1, :], pouts_[h][0][0 : DK + 1, :]
                            )
                        # Z = raw row 0; 1/Z broadcast across partitions 0-64
                        rzf = normp.tile([128, 512], f32, tag="rzf", name="rzf_t")
                        nc.vector.reciprocal_approx_fast(rzf[0:1, :], raw[0:1, :])
                        rzbc = normp.tile([128, 512], f32, tag="rzbc", name="rzbc_t")
                        nc.gpsimd.partition_broadcast(
                            rzbc[0 : DK + 1, :], rzf[0:1, :]
                        )
                        ot = outp.tile([128, 512], f32, tag="ot", name="ot_t")
                        nc.vector.tensor_mul(
                            ot[0 : DK + 1, :], raw[0 : DK + 1, :], rzbc[0 : DK + 1, :]
                        )
                        nc.sync.dma_start(
                            out=outT.ap()[h * DK : (h + 1) * DK, n0_ : n0_ + 512],
                            in_=ot[1 : DK + 1, :],
                        )

                npo = 2 if CFG["pv_split"] else 1
                tix = 0
                for nch in range(NCH):
                    n0 = nch * 512
                    for mt in range(MT):
                        if prefetch and tix < 48:
                            t = "kvq"[tix // 16]
                            emit_xt_dma(xt_cur[t], t, tix % 16)
                        tix += 1
                        if mt == 0:
                            state["pouts"] = [
                                [
                                    popool.tile(
                                        [128, 512], f32, tag="po",
                                        name=f"pout{nch}_{h}_{i}",
                                    )
                                    for i in range(npo)
                                ]
                                for h in range(HPC)
                            ]
                        pouts = state["pouts"]
                        bt = biasp.tile([128, HPC, 512], bias_dt, tag="bt", name="ebk_t")
                        bta = ebkT.ap()
                        nc.sync.dma_start(
                            out=bt,
                            in_=bass.AP(
                                tensor=bta.tensor,
                                offset=bta.offset + mt * 128 * N + n0,
                                ap=[[N, 128], [N * N, HPC], [1, 512]],
                            ),
                        )
                        pss = [
                            pspool.tile([128, 512], f32, tag="ps", name=f"score_ps{h}")
                            for h in range(HPC)
                        ]
                        et = ep.tile([128, 1024], bf16, tag="et", name="e_t")
                        et3 = et.rearrange("p (h n) -> p h n", h=HPC)
                        # scores^T: kh @ qh^T, K=64 per head; the two heads sit
                        # at base partitions 0/64 -> concurrent PE row-tiles
                        for h in range(HPC):
                            nc.tensor.matmul(
                                pss[h],
                                lhsT=proj["k"][
                                    h * DK : (h + 1) * DK, mt * 128 : (mt + 1) * 128
                                ],
                                rhs=proj["q"][h * DK : (h + 1) * DK, n0 : n0 + 512],
                                start=True,
                                stop=True,
                            )
                        for h in range(HPC):
                            nc.scalar.activation(et3[:, h, :], pss[h], AF.Exp)
                        # bias product + explored mask in one fused DVE op
                        if CFG["bias_fp8"]:
                            nc.vector._custom_dve(
                                mm_op, out=et3, in0=et3, in1=bt, s0=-1.0
                            )
                        else:
                            nc.vector._custom_dve(mm_op, out=et3, in0=et3, in1=bt)
                        pending.append((et3, mt, n0, pouts))
                        if len(pending) > CFG["pending_depth"]:
                            emit_pv(pending.pop(0))
                while pending:
                    emit_pv(pending.pop(0))

            if repeat == 1:
                emit_body(xt0, prefetch=False)
            else:
                unroll = 4 if repeat % 4 == 0 else 2
                assert repeat % unroll == 0, "timing build unrolls bodies per For_i"
                with tc.For_i(
                    0,
                    repeat // unroll,
                    1,
                    hint_engines=(
                        mybir.EngineType.PE,
                        mybir.EngineType.DVE,
                        mybir.EngineType.Activation,
                        mybir.EngineType.Pool,
                        mybir.EngineType.SP,
                    ),
                ):
                    for _ in range(unroll):
                        emit_body(xt0, prefetch=True)

    nc.compile()
    return nc


def _wlayout(w):
    # [HID, DC] -> [128, KT*DC]: partition-major k-tile layout, contiguous DMA
    return np.ascontiguousarray(
        w.reshape(KT, 128, DC).transpose(1, 0, 2).reshape(128, KT * DC)
    ).astype(BF16)


def stage_inputs(q, k, v, attn_bias, explored, Wq, bq, Wk, bk, Wv, bv):
    """Host-side sharding/layout staging. Returns in_maps for 8 cores."""
    scale = DK ** -0.5
    xT = {
        "q": np.ascontiguousarray(np.asarray(q, np.float32).T).astype(BF16),
        "k": np.ascontiguousarray(np.asarray(k, np.float32).T).astype(BF16),
        "v": np.ascontiguousarray(np.asarray(v, np.float32).T).astype(BF16),
    }
    Wq = np.asarray(Wq, np.float32) * scale
    bq = np.asarray(bq, np.float32) * scale
    Wk = np.asarray(Wk, np.float32)
    bk = np.asarray(bk, np.float32)
    Wv = np.asarray(Wv, np.float32)
    bv = np.asarray(bv, np.float32)
    attn_bias = np.asarray(attn_bias, np.float32)
    explored = np.asarray(explored)

    # keep mask, transposed: [key m, query n]; row/col 0 always kept
    keepT = np.ones((N, N), dtype=bool)
    keepT[1:, 1:] = (explored != 0).T

    in_maps = []
    for c in range(NCORES):
        cols = slice(c * DC, (c + 1) * DC)
        h0 = HPC * c
        eb = np.exp(attn_bias[h0 : h0 + HPC].transpose(0, 2, 1))
        if CFG["bias_fp8"]:
            # EBK-1 in fp8e4m3; masked entries exactly -1.0
            ebk = np.where(keepT[None, :, :], eb - 1.0, -1.0).astype(FP8)
        else:
            ebk = (eb * keepT[None, :, :]).astype(BF16)
        ebk = ebk.reshape(HPC * N, N)
        in_maps.append(
            {
                "xTq": xT["q"],
                "xTk": xT["k"],
                "xTv": xT["v"],
                "Wq": _wlayout(Wq[:, cols]),
                "Wk": _wlayout(Wk[:, cols]),
                "Wv": _wlayout(Wv[:, cols]),
                "bq": bq[cols].reshape(DC, 1).copy(),
                "bk": bk[cols].reshape(DC, 1).copy(),
                "bv": bv[cols].reshape(DC, 1).copy(),
                "ebkT": ebk,
            }
        )
    return in_maps


def assemble_output(results):
    """results: list of 8 dicts with 'outT' [128, 2048] f32."""
    out = np.empty((N, HEADS * DK), dtype=np.float32)
    for c in range(NCORES):
        r = np.asarray(results[c]["outT"])
        for j in range(HPC):
            h = HPC * c + j
            out[:, h * DK : (h + 1) * DK] = r[j * DK : (j + 1) * DK, :].T
    return out


def get_compiled(repeat=1):
    key = ("nc", repeat, tuple(sorted(CFG.items())))
    if key not in _cache:
        _cache[key] = _build(repeat)
    return _cache[key]


def kernel(**inputs) -> np.ndarray:
    from concourse.bass_utils import run_bass_kernel_spmd

    nc = get_compiled()
    in_maps = stage_inputs(**inputs)
    res = run_bass_kernel_spmd(nc, in_maps, core_ids=list(range(NCORES)))
    return assemble_output(res.results)


# revision 22
# speedup vs baseline: 1.0859x; 1.0859x over previous
"""Trainium2 Bass kernel for nn_MultiHeadAttention_26259430048704.

Multi-head attention with additive bias and a multiplicative "explored" mask
applied to the scores before softmax (masked scores are set to 0, so they
contribute exp(0)=1 to the softmax).

Sharding: 16 heads / 8 cores = 2 heads per core (tensor parallel over heads).
Each core computes projections for its 128 W-columns and full attention for
its 2 heads; the host concatenates the per-core transposed outputs.

Kernel structure (v4):
  - bias folded multiplicatively on the host: EBKm1 = keep*exp(bias^T) - 1
    in fp8e4m3 (masked entries = -1.0 exactly).  On device:
    et = exp(qk); et = et*(EBKm1+1) + (EBKm1==-1)  via ONE custom fused DVE
    op.  No identity-bias matmuls on the PE, no u8 mask / copy_predicated,
    and the bias stream is 1 byte/elem.
  - QK is K=64 per head at base partitions 0/64 -> the two heads run as
    concurrent PE row-tiles.  pV is likewise split into two concurrent K=64
    row-tiles per head, accumulating in separate psum banks merged by the
    normalization (ACT copy + DVE add).
  - softmax denominator: vhp column 0 is ones so Z = pout row 0; 1/Z via
    DVE reciprocal_approx_fast, replicated across partitions by gpsimd
    partition_broadcast (idle engine), then one DVE multiply.
  - The x^T inputs live in three persistent SBUF tiles; each loop iteration
    re-DMAs all 12MB for the NEXT iteration, interleaved one 256KB chunk per
    attention tile, so the bulk load never queues ahead of the per-tile EBK
    DMAs (which starved the pipeline by ~30us/iter when batched).
  - The timing build unrolls TWO bodies per For_i iteration so double
    buffered pools (projections, vhp) truly ping-pong across iterations
    (static emission pins pool slots; an unrolled pair alternates them).
"""

import sys

for _p in ("/opt/trn_rl_repo", "/root/.axon_site/_ro/trn_rl_repo"):
    if _p not in sys.path:
        sys.path.insert(0, _p)

import numpy as np
import ml_dtypes

BF16 = ml_dtypes.bfloat16
FP8 = ml_dtypes.float8_e4m3

N = 2048
HID = 1024
HEADS = 16
DK = 64
NCORES = 8
HPC = HEADS // NCORES  # 2 heads per core
DC = HPC * DK  # 128 output columns per core
KT = HID // 128  # 8 contraction tiles
MT = N // 128  # 16 m tiles
NCH = N // 512  # 4 n chunks

_cache = {}

CFG = {
    "pv_split": True,      # split pV into two concurrent K=64 row-tiles
    "proj_on_act": True,   # projection bias-add on ACT (else DVE)
    "vhp_on_act": True,    # vhp extraction copies on ACT (else DVE)
    "pending_depth": 5,    # software pipeline depth for pV
    "bias_fp8": True,      # EBK-1 in fp8e4m3 (else EBK in bf16)
}

_OPS_REGISTERED = {}


def _register_op(name, spec):
    import concourse.dve_ops as dve_ops
    from concourse.dve_ops import DveOp
    from concourse.dve_table_gen import dve_ver_for
    from concourse.dve_uop import DveOpSpec
    from concourse.dve_spec import lower

    if name in _OPS_REGISTERED:
        return _OPS_REGISTERED[name]
    for op in dve_ops.OPS:
        if op.name == name:
            _OPS_REGISTERED[name] = op
            return op
    ver = dve_ver_for("TRN2")
    opcode = max(dve_ops._SUB_OPCODE_FOR_NAME.values()) + 1
    sha = DveOpSpec(
        name=name, opcode=opcode, uops=lower(spec, ver=ver), rd1_en=True
    ).sha(ver)
    op = DveOp(name, spec, subdim=False, uops_sha={ver: sha})
    dve_ops.OPS.append(op)
    dve_ops.CUSTOM_DVE_SPECS[name] = spec
    dve_ops._SUB_OPCODE_FOR_NAME[name] = opcode
    _OPS_REGISTERED[name] = op
    return op


def _get_masked_mul_op():
    """bf16 variant: out = in0*in1 + (in1==0)."""
    from concourse.dve_spec import Spec, Src0, Src1, Zero, eq

    def _ref(in0, in1, c0, c1, c2):
        a = np.asarray(in0, np.float32)
        b = np.asarray(in1, np.float32)
        return a * b + (b == 0).astype(np.float32)

    return _register_op("MASKED_MUL_ANT", Spec(body=Src0 * Src1 + eq(Src1, Zero), reference=_ref))


def _get_masked_mul_m1_op():
    """fp8 variant: out = in0*(in1+1) + (in1==s0); called with s0=-1."""
    from concourse.dve_spec import Spec, Src0, Src1, One, C0, eq

    def _ref(in0, in1, c0, c1, c2):
        a = np.asarray(in0, np.float32)
        b = np.asarray(in1, np.float32)
        c0v = c0 if isinstance(c0, float) else np.asarray(c0, np.float32)
        return a * (b + 1.0) + (b == c0v).astype(np.float32)

    return _register_op(
        "MASKED_MUL_M1_ANT", Spec(body=Src0 * (Src1 + One) + eq(Src1, C0), reference=_ref)
    )


def _build(repeat=1):
    import concourse.bass as bass
    import concourse.bacc as bacc
    import concourse.mybir as mybir
    import concourse.tile as tile
    from concourse.masks import make_identity

    f32 = mybir.dt.float32
    bf16 = mybir.dt.bfloat16
    AF = mybir.ActivationFunctionType
    bias_dt = mybir.dt.float8e4 if CFG["bias_fp8"] else bf16
    mm_op = _get_masked_mul_m1_op() if CFG["bias_fp8"] else _get_masked_mul_op()

    nc = bacc.Bacc("TRN2", target_bir_lowering=False, debug=False)

    xts = {t: nc.dram_tensor(f"xT{t}", [HID, N], bf16, kind="ExternalInput") for t in "qkv"}
    Ws = {t: nc.dram_tensor(f"W{t}", [128, KT * DC], bf16, kind="ExternalInput") for t in "qkv"}
    bs = {t: nc.dram_tensor(f"b{t}", [DC, 1], f32, kind="ExternalInput") for t in "qkv"}
    ebkT = nc.dram_tensor("ebkT", [HPC * N, N], bias_dt, kind="ExternalInput")
    outT = nc.dram_tensor("outT", [DC, N], f32, kind="ExternalOutput")

    with tile.TileContext(nc) as tc:
        with (
            tc.tile_pool(name="constp", bufs=1) as constp,
            tc.tile_pool(name="xtp", bufs=1) as xtp,
            tc.tile_pool(name="pers", bufs=2) as pers,
            tc.tile_pool(name="biasp", bufs=16) as biasp,
            tc.tile_pool(name="ep", bufs=8) as ep,
            tc.tile_pool(name="normp", bufs=4) as normp,
            tc.tile_pool(name="outp", bufs=4) as outp,
            tc.tile_pool(name="projp", bufs=1, space="PSUM") as projp,
            tc.tile_pool(name="pspool", bufs=3, space="PSUM") as pspool,
            tc.tile_pool(name="popool", bufs=4, space="PSUM") as popool,
        ):
            ident = constp.tile([128, 128], bf16)
            make_identity(nc, ident)

            W_sb = {}
            b_sb = {}
            for t in "kvq":
                W_sb[t] = constp.tile([128, KT, DC], bf16, tag=f"w{t}", name=f"W{t}_sb")
                nc.sync.dma_start(
                    out=W_sb[t], in_=Ws[t].ap().rearrange("p (kt m) -> p kt m", kt=KT)
                )
                b_sb[t] = constp.tile([DC, 1], f32, tag=f"b{t}", name=f"b{t}_sb")
                nc.sync.dma_start(out=b_sb[t], in_=bs[t].ap())

            def emit_xt_dma(tile_, t, chunk):
                kt, ch2 = divmod(chunk, 2)
                xt_dram = xts[t].ap().rearrange("(kt p) n -> p kt n", p=128)
                nc.sync.dma_start(
                    out=tile_[:, kt, ch2 * 1024 : (ch2 + 1) * 1024],
                    in_=xt_dram[:, kt, ch2 * 1024 : (ch2 + 1) * 1024],
                )

            # persistent x^T tiles: the prologue fills them; each body re-DMAs
            # them in place (write-after-read deps keep this safe) so the 12MB
            # stream overlaps the previous body's attention phase.
            xt0 = {}
            for t in "kvq":
                xt0[t] = xtp.tile([128, KT, N], bf16, tag=f"xt{t}", name=f"xt_{t}")
                for c in range(16):
                    emit_xt_dma(xt0[t], t, c)

            def emit_body(xt_cur, prefetch):
                """One full kernel pass; reads xt_cur, optionally emits the
                next iteration's xt DMAs interleaved into the attention loop.
                Returns the tiles holding the next iteration's x^T."""
                # ---- Phase 1: projections ----
                proj = {}
                for t in "kvq":
                    proj[t] = pers.tile([128, N], bf16, tag=f"proj{t}", name=f"proj{t}_sb")
                    for ch in range(NCH):
                        ps = projp.tile([128, 512], f32, tag="pp", name="proj_ps")
                        for kt in range(KT):
                            nc.tensor.matmul(
                                ps,
                                lhsT=W_sb[t][:, kt, :],
                                rhs=xt_cur[t][:, kt, ch * 512 : (ch + 1) * 512],
                                start=(kt == 0),
                                stop=(kt == KT - 1),
                            )
                        if CFG["proj_on_act"]:
                            nc.scalar.activation(
                                proj[t][:, ch * 512 : (ch + 1) * 512],
                                ps,
                                AF.Identity,
                                bias=b_sb[t],
                            )
                        else:
                            nc.vector.tensor_scalar_add(
                                proj[t][:, ch * 512 : (ch + 1) * 512], ps, b_sb[t]
                            )

                # vh' per head: [m-part, mt, 65]; col 0 = ones (softmax denom
                # lands on psum partition 0, where gpsimd can broadcast from)
                vhp = [
                    pers.tile([128, MT, DK + 1], bf16, tag=f"vhp{h}", name=f"vhp{h}_sb")
                    for h in range(HPC)
                ]
                for h in range(HPC):
                    nc.vector.memset(vhp[h][:, :, 0:1], 1.0)
                for mb in range(MT):
                    pstr = projp.tile([128, 128], bf16, tag="pp", name="tr_ps")
                    nc.tensor.transpose(
                        pstr, proj["v"][:, mb * 128 : (mb + 1) * 128], ident
                    )
                    for h in range(HPC):
                        if CFG["vhp_on_act"]:
                            nc.scalar.copy(
                                vhp[h][:, mb, 1 : DK + 1], pstr[:, h * DK : (h + 1) * DK]
                            )
                        else:
                            nc.vector.tensor_copy(
                                vhp[h][:, mb, 1 : DK + 1], pstr[:, h * DK : (h + 1) * DK]
                            )

                # ---- Phase 2: attention ----
                state = {"pouts": None}
                pending = []

                def emit_pv(item):
                    et3, m, n0_, pouts_ = item
                    for h in range(HPC):
                        if CFG["pv_split"]:
                            nc.tensor.matmul(
                                pouts_[h][0][0 : DK + 1, :],
                                lhsT=vhp[h][0:64, m, :],
                                rhs=et3[0:64, h, :],
                                start=(m == 0),
                                stop=(m == MT - 1),
                            )
                            nc.tensor.matmul(
                                pouts_[h][1][0 : DK + 1, :],
                                lhsT=vhp[h][64:128, m, :],
                                rhs=et3[64:128, h, :],
                                start=(m == 0),
                                stop=(m == MT - 1),
                            )
                        else:
                            nc.tensor.matmul(
                                pouts_[h][0][0 : DK + 1, :],
                                lhsT=vhp[h][:, m, :],
                                rhs=et3[:, h, :],
                                start=(m == 0),
                                stop=(m == MT - 1),
                            )
                    if m == MT - 1:
                        emit_norm(n0_, pouts_)

                def emit_norm(n0_, pouts_):
                    for h in range(HPC):
                        raw = normp.tile([128, 512], f32, tag="raw", name="raw_t")
                        if CFG["pv_split"]:
                            # DVE may read only one PSUM operand: stage half A
                            # through the ACT engine, then add half B on DVE.
                            nc.scalar.copy(
                                raw[0 : DK + 1, :], pouts_[h][0][0 : DK + 1, :]
                            )
                            nc.vector.tensor_add(
                                raw[0 : DK + 1, :],
                                raw[0 : DK + 1, :],
                                pouts_[h][1][0 : DK + 1, :],
                            )
                        else:
                            nc.vector.tensor_copy(
                                raw[0 : DK + 1, :], pouts_[h][0][0 : DK + 1, :]
                            )
                        # Z = raw row 0; 1/Z broadcast across partitions 0-64
                        rzf = normp.tile([128, 512], f32, tag="rzf", name="rzf_t")
                        nc.vector.reciprocal_approx_fast(rzf[0:1, :], raw[0:1, :])
                        rzbc = normp.tile([128, 512], f32, tag="rzbc", name="rzbc_t")
                        nc.gpsimd.partition_broadcast(
                            rzbc[0 : DK + 1, :], rzf[0:1, :]
                        )
                        ot = outp.tile([128, 512], f32, tag="ot", name="ot_t")
                        nc.vector.tensor_mul(
                            ot[0 : DK + 1, :], raw[0 : DK + 1, :], rzbc[0 : DK + 1, :]
                        )
                        nc.sync.dma_start(
                            out=outT.ap()[h * DK : (h + 1) * DK, n0_ : n0_ + 512],
                            in_=ot[1 : DK + 1, :],
                        )

                npo = 2 if CFG["pv_split"] else 1
                tix = 0
                for nch in range(NCH):
                    n0 = nch * 512
                    for mt in range(MT):
                        if prefetch and tix < 48:
                            t = "kvq"[tix // 16]
                            emit_xt_dma(xt_cur[t], t, tix % 16)
                        tix += 1
                        if mt == 0:
                            state["pouts"] = [
                                [
                                    popool.tile(
                                        [128, 512], f32, tag="po",
                                        name=f"pout{nch}_{h}_{i}",
                                    )
                                    for i in range(npo)
                                ]
                                for h in range(HPC)
                            ]
                        pouts = state["pouts"]
                        bt = biasp.tile([128, HPC, 512], bias_dt, tag="bt", name="ebk_t")
                        bta = ebkT.ap()
                        nc.sync.dma_start(
                            out=bt,
                            in_=bass.AP(
                                tensor=bta.tensor,
                                offset=bta.offset + mt * 128 * HPC * N + n0,
                                ap=[[HPC * N, 128], [N, HPC], [1, 512]],
                            ),
                        )
                        pss = [
                            pspool.tile([128, 512], f32, tag="ps", name=f"score_ps{h}")
                            for h in range(HPC)
                        ]
                        et = ep.tile([128, 1024], bf16, tag="et", name="e_t")
                        et3 = et.rearrange("p (h n) -> p h n", h=HPC)
                        # scores^T: kh @ qh^T, K=64 per head; the two heads sit
                        # at base partitions 0/64 -> concurrent PE row-tiles
                        for h in range(HPC):
                            nc.tensor.matmul(
                                pss[h],
                                lhsT=proj["k"][
                                    h * DK : (h + 1) * DK, mt * 128 : (mt + 1) * 128
                                ],
                                rhs=proj["q"][h * DK : (h + 1) * DK, n0 : n0 + 512],
                                start=True,
                                stop=True,
                            )
                        for h in range(HPC):
                            nc.scalar.activation(et3[:, h, :], pss[h], AF.Exp)
                        # bias product + explored mask in one fused DVE op
                        if CFG["bias_fp8"]:
                            nc.vector._custom_dve(
                                mm_op, out=et3, in0=et3, in1=bt, s0=-1.0
                            )
                        else:
                            nc.vector._custom_dve(mm_op, out=et3, in0=et3, in1=bt)
                        pending.append((et3, mt, n0, pouts))
                        if len(pending) > CFG["pending_depth"]:
                            emit_pv(pending.pop(0))
                while pending:
                    emit_pv(pending.pop(0))

            if repeat == 1:
                emit_body(xt0, prefetch=False)
            else:
                unroll = 4 if repeat % 4 == 0 else 2
                assert repeat % unroll == 0, "timing build unrolls bodies per For_i"
                with tc.For_i(
                    0,
                    repeat // unroll,
                    1,
                    hint_engines=(
                        mybir.EngineType.PE,
                        mybir.EngineType.DVE,
                        mybir.EngineType.Activation,
                        mybir.EngineType.Pool,
                        mybir.EngineType.SP,
                    ),
                ):
                    for _ in range(unroll):
                        emit_body(xt0, prefetch=True)

    nc.compile()
    return nc


def _wlayout(w):
    # [HID, DC] -> [128, KT*DC]: partition-major k-tile layout, contiguous DMA
    return np.ascontiguousarray(
        w.reshape(KT, 128, DC).transpose(1, 0, 2).reshape(128, KT * DC)
    ).astype(BF16)


def stage_inputs(q, k, v, attn_bias, explored, Wq, bq, Wk, bk, Wv, bv):
    """Host-side sharding/layout staging. Returns in_maps for 8 cores."""
    scale = DK ** -0.5
    xT = {
        "q": np.ascontiguousarray(np.asarray(q, np.float32).T).astype(BF16),
        "k": np.ascontiguousarray(np.asarray(k, np.float32).T).astype(BF16),
        "v": np.ascontiguousarray(np.asarray(v, np.float32).T).astype(BF16),
    }
    Wq = np.asarray(Wq, np.float32) * scale
    bq = np.asarray(bq, np.float32) * scale
    Wk = np.asarray(Wk, np.float32)
    bk = np.asarray(bk, np.float32)
    Wv = np.asarray(Wv, np.float32)
    bv = np.asarray(bv, np.float32)
    attn_bias = np.asarray(attn_bias, np.float32)
    explored = np.asarray(explored)

    # keep mask, transposed: [key m, query n]; row/col 0 always kept
    keepT = np.ones((N, N), dtype=bool)
    keepT[1:, 1:] = (explored != 0).T

    in_maps = []
    for c in range(NCORES):
        cols = slice(c * DC, (c + 1) * DC)
        h0 = HPC * c
        eb = np.exp(attn_bias[h0 : h0 + HPC].transpose(0, 2, 1))
        if CFG["bias_fp8"]:
            # EBK-1 in fp8e4m3; masked entries exactly -1.0
            ebk = np.where(keepT[None, :, :], eb - 1.0, -1.0).astype(FP8)
        else:
            ebk = (eb * keepT[None, :, :]).astype(BF16)
        # interleave heads per key row -> 1KB contiguous DMA lines
        ebk = np.ascontiguousarray(ebk.transpose(1, 0, 2)).reshape(HPC * N, N)
        in_maps.append(
            {
                "xTq": xT["q"],
                "xTk": xT["k"],
                "xTv": xT["v"],
                "Wq": _wlayout(Wq[:, cols]),
                "Wk": _wlayout(Wk[:, cols]),
                "Wv": _wlayout(Wv[:, cols]),
                "bq": bq[cols].reshape(DC, 1).copy(),
                "bk": bk[cols].reshape(DC, 1).copy(),
                "bv": bv[cols].reshape(DC, 1).copy(),
                "ebkT": ebk,
            }
        )
    return in_maps


def assemble_output(results):
    """results: list of 8 dicts with 'outT' [128, 2048] f32."""
    out = np.empty((N, HEADS * DK), dtype=np.float32)
    for c in range(NCORES):
        r = np.asarray(results[c]["outT"])
        for j in range(HPC):
            h = HPC * c + j
            out[:, h * DK : (h + 1) * DK] = r[j * DK : (j + 1) * DK, :].T
    return out


def get_compiled(repeat=1):
    key = ("nc", repeat, tuple(sorted(CFG.items())))
    if key not in _cache:
        _cache[key] = _build(repeat)
    return _cache[key]


def kernel(**inputs) -> np.ndarray:
    from concourse.bass_utils import run_bass_kernel_spmd

    nc = get_compiled()
    in_maps = stage_inputs(**inputs)
    res = run_bass_kernel_spmd(nc, in_maps, core_ids=list(range(NCORES)))
    return assemble_output(res.results)
